# revision 14
# baseline (speedup 1.0000x reference)
"""DWA LanguageModel layer on 8 trn2 NeuronCores (v4).

Strategy (changes vs v3):
  - Keys path is latency-critical (it gates the AllGather): pk/wk are
    DMA'd as 8 interleaved 256KB chunk pairs and the 128 keys matmuls
    chase the chunks, so the collective triggers at ~22us instead of
    50us.  PE is pre-warmed with dummy matmuls so keys run at 2.4GHz.
  - AllGather output lives in addr_space="Shared" HBM (direct mesh
    writes) and the knT load-back goes on the sync ring (ACT ring
    stays clear for the alpha-phase activations).
  - The dynamic path computes t TRANSPOSED pre-collective (tT[nr,b]
    via 256 small matmuls under the AG shadow) with pool rows
    reordered (r,n) host-side, so after alpha only 8 elementwise
    mults (sT = tT * alphaT, fp8 out) and a clean DoubleRow fp8
    matmul stream (40 MMs, contraction 256) remain on the tail.
  - alpha chain runs in bf16 from SBUF scores; ACT tables are
    preloaded during the collective dead-zone; LN is fused via
    accum_out row-sums and per-partition scalar chains.
  - One flat PSUM pool, bank-exact tiles, no cross-pool reuse (the
    tile allocator's released-zone deps were racy for this shape).
"""
import sys

sys.path.insert(0, "/opt/trn_rl_repo")
import numpy as np
import ml_dtypes

import concourse.bass as bass
import concourse.mybir as mybir
import concourse.tile as tile
from concourse import bacc
from concourse.bass_utils import run_bass_kernel_spmd
from concourse.masks import make_identity

F32 = mybir.dt.float32
BF16 = mybir.dt.bfloat16
FP8 = mybir.dt.float8e4
AF = mybir.ActivationFunctionType
ALU = mybir.AluOpType
DR = mybir.MatmulPerfMode.DoubleRow

NCORES = 8
B = 1024            # tokens
BL = B // NCORES    # tokens per core = 128
D_MODEL = 512
N = 1024            # pool rows
NL = N // NCORES    # pool rows per core = 128
D = 16384           # pool cols
S = 2
DK = 64
SDK = S * DK        # 128
R = 8
NR = N * R          # 8192
K_MAX = 16
LAMBDA_SHARP = 5.0
LN_EPS = 1e-5
U_END = D_MODEL * R          # 4096
V_END = U_END + R * D_MODEL  # 8192
B_END = V_END + D_MODEL      # 8704

SC_V = 16.0          # scale on V^T
SC_U = 64.0          # scale on U_perm
SC_B = SC_V * SC_U   # scale on bias rows (alpha chunk is unscaled)
SC_H = SC_V * SC_U   # total scale on h1 psum = 1024

NKCH = 8             # pk/wk DMA chunks
SCORES_AFTER_R = 5   # emit scores matmuls after this many tT r-groups
LAST_EXEC_NS = None
TRACE = False
TRACE_CORES = None
TMPDIR = None
NO_CC = False
SHARED_CC = True
USE_DR = True


def _build(tau_f, w0_f, w1_f, gamma_f):
    nc = bacc.Bacc("TRN2", target_bir_lowering=False, debug=False,
                   num_devices=NCORES)

    # ---- I/O (all pre-packed to exact SBUF layout [128, X]) ----
    KCW = (128 // NKCH) * NL                # cols per pk/wk chunk
    pk_d = [nc.dram_tensor(f"pk{c}", [128, KCW], FP8, kind="ExternalInput")
            for c in range(NKCH)]
    wk_d = [nc.dram_tensor(f"wk{c}", [128, KCW], FP8, kind="ExternalInput")
            for c in range(NKCH)]
    wq_d = nc.dram_tensor("wq", [128, 4 * SDK], BF16, kind="ExternalInput")
    zt_d = nc.dram_tensor("zt", [128, 4 * BL], FP8, kind="ExternalInput")
    zb_d = nc.dram_tensor("zb", [BL, D_MODEL], F32, kind="ExternalInput")
    ls_d = nc.dram_tensor("ls", [BL, D_MODEL], F32, kind="ExternalInput")
    lb_d = nc.dram_tensor("lb", [BL, D_MODEL], F32, kind="ExternalInput")
    wbt_d = nc.dram_tensor("wbt", [128, 4 * D_MODEL], BF16,
                           kind="ExternalInput")
    vt_ds = [nc.dram_tensor(f"vt{r}", [128, 4 * N], FP8,
                            kind="ExternalInput") for r in range(R)]
    up_ds = [nc.dram_tensor(f"up{q}", [128, 18 * D_MODEL], FP8,
                            kind="ExternalInput") for q in range(4)]
    out_d = nc.dram_tensor("out", [BL, D_MODEL], F32, kind="ExternalOutput")

    with tile.TileContext(nc) as tc:
        with (
            tc.tile_pool(name="sb", bufs=1) as sb,
            tc.tile_pool(name="ps", bufs=1, space="PSUM") as ps,
        ):
            _emit(nc, tc, sb, ps, tau_f, w0_f, w1_f, gamma_f,
                  pk_d, wk_d, wq_d, zt_d, zb_d, ls_d, lb_d, wbt_d,
                  vt_ds, up_ds, out_d, KCW)

    nc.compile()
    return nc


def _emit(nc, tc, sb, ps, tau_f, w0_f, w1_f, gamma_f,
          pk_d, wk_d, wq_d, zt_d, zb_d, ls_d, lb_d, wbt_d,
          vt_ds, up_ds, out_d, KCW):
    KPC = 128 // NKCH  # keys matmuls per chunk

    # ---------- DMA issues, priority order ----------
    pk_sb, wk_sb = [], []
    for ch in range(NKCH):
        pt = sb.tile([128, KCW], FP8, tag=f"pk{ch}")
        wt = sb.tile([128, KCW], FP8, tag=f"wk{ch}")
        nc.sync.dma_start(pt[:], pk_d[ch][:])
        nc.sync.dma_start(wt[:], wk_d[ch][:])
        pk_sb.append(pt)
        wk_sb.append(wt)
    zt_sb = sb.tile([128, 4 * BL], FP8, tag="zt")
    wq_sb = sb.tile([128, 4 * SDK], BF16, tag="wq")
    wbt_sb = sb.tile([128, 4 * D_MODEL], BF16, tag="wbt")
    zb_sb = sb.tile([BL, D_MODEL], F32, tag="zb")
    vt_sb = [sb.tile([128, 4 * N], FP8, tag=f"vt{r}", name=f"vt{r}")
             for r in range(R)]
    up_sb = [sb.tile([128, 18 * D_MODEL], FP8, tag=f"up{q}", name=f"up{q}")
             for q in range(4)]
    ls_sb = sb.tile([BL, D_MODEL], F32, tag="ls")
    lb_sb = sb.tile([BL, D_MODEL], F32, tag="lb")

    def emit_late_input_dmas():
        # emitted after cc_in so the collective trigger's completion-sem
        # lane only aliases early pk/wk chunk loads
        nc.sync.dma_start(zt_sb[:], zt_d[:])
        nc.sync.dma_start(wq_sb[:], wq_d[:])
        nc.sync.dma_start(wbt_sb[:], wbt_d[:])
        nc.sync.dma_start(zb_sb[:], zb_d[:])
        for r in range(R):
            nc.sync.dma_start(vt_sb[r][:], vt_ds[r][:])
        for q in range(4):
            nc.sync.dma_start(up_sb[q][:], up_ds[q][:])
        nc.sync.dma_start(ls_sb[:], ls_d[:])
        nc.sync.dma_start(lb_sb[:], lb_d[:])

    def up_ap(c0, ncols):
        """columns [c0*512, (c0+ncols)*512) of the packed up matrix"""
        q, j = divmod(c0, 18)
        assert j + ncols <= 18
        return up_sb[q][:, j * D_MODEL:(j + ncols) * D_MODEL]

    # ---------- PSUM layout: one flat pool, 8 banks, no reuse ----------
    # scores_big doubles as keys/q/h2 accumulators (regions die before
    # the scores matmuls write) so the alpha chain reads PSUM directly
    scores_big = ps.tile([128, 1024], F32, tag="scores_big")
    keys_ps = scores_big[:, 0:SDK]
    q_ps = scores_big[:, SDK:2 * SDK]
    h2_ps = scores_big[:, 512:1024]
    tp_ps = ps.tile([128, 1024], BF16, tag="tp")    # kn_tp|q_tp|a_tp x8
    kn_tp = tp_ps[:, 0:NL]
    q_tp = tp_ps[:, NL:2 * NL]
    h1_ps = ps.tile([BL, D_MODEL], F32, tag="h1")

    # ---------- dummy collective: absorb the CC-stream barrier early ----
    # ---------- early engine warmup ----------
    warm_sb = sb.tile([128, 128], BF16, tag="warm")
    nc.vector.memset(warm_sb[:], 0.0)
    identb = sb.tile([128, 128], BF16, tag="identb")
    make_identity(nc, identb[:])
    # preload ACT sqrt table while DMAs stream (scheduler hoists this)
    dum_in = sb.tile([1, 1], F32, tag="dum_in")
    dum_out = sb.tile([1, 8], F32, tag="dum_out")
    nc.vector.memset(dum_in[:], 1.0)
    nc.scalar.activation(dum_out[:, 0:1], dum_in[:], AF.Sqrt)

    tT_sb = sb.tile([BL, NR], BF16, tag="tT")
    sT_sb = sb.tile([BL, NR], FP8, tag="sT")
    q_n = sb.tile([BL, SDK], BF16, tag="q_n")
    qnT = sb.tile([SDK, BL], BF16, tag="qnT")
    hz_sb = sb.tile([BL, D_MODEL], F32, tag="hz")     # zb + gamma*h2
    knTl = sb.tile([SDK, NL], BF16, tag="knTl")
    knT = sb.tile([SDK, N], BF16, tag="knT")
    alphaT = sb.tile([128, 8 * BL], FP8, tag="alphaT")

    for i in range(24):
        nc.tensor.matmul(keys_ps[:], warm_sb[:], warm_sb[:],
                         start=(i == 0), stop=(i == 23))

    # ---------- keys for local 128 pool rows, DMA-chased ----------
    for k in range(128):
        ch, kk = divmod(k, KPC)
        nc.tensor.matmul(keys_ps[:],
                         pk_sb[ch][:, kk * NL:(kk + 1) * NL],
                         wk_sb[ch][:, kk * SDK:(kk + 1) * SDK],
                         start=(k == 0), stop=(k == 127))
    # normalize along free dim per aspect
    ksq = sb.tile([NL, S], F32, tag="ksq")
    ksc = sb.tile([NL, SDK], F32, tag="sqscr")
    for s in range(S):
        nc.scalar.activation(ksc[:, s * DK:(s + 1) * DK],
                             keys_ps[:, s * DK:(s + 1) * DK],
                             AF.Square,
                             accum_out=ksq[:, s:s + 1])
    knorm = sb.tile([NL, S], F32, tag="knorm")
    nc.scalar.activation(knorm[:], ksq[:], AF.Sqrt)
    krec = sb.tile([NL, S], F32, tag="krec")
    nc.vector.tensor_scalar_add(knorm[:], knorm[:], 1e-8)
    nc.vector.reciprocal(krec[:], knorm[:])
    kn_w = sb.tile([NL, SDK], BF16, tag="kn_w")
    for s in range(S):
        nc.vector.tensor_scalar(
            kn_w[:, s * DK:(s + 1) * DK],
            keys_ps[:, s * DK:(s + 1) * DK],
            krec[:, s:s + 1], None, op0=ALU.mult)
    nc.tensor.transpose(kn_tp, kn_w[:], identb[:])
    nc.scalar.activation(knTl[:], kn_tp, AF.Copy)

    # ---------- AllGather normalized keysT (bf16, 32KB in) ----------
    cc_in_t = nc.dram_tensor("cc_in", [SDK, NL], BF16, kind="Internal")
    cc_out_t = nc.dram_tensor(
        "cc_out", [N, NL], BF16, kind="Internal",
        addr_space="Shared" if (SHARED_CC and not NO_CC) else "Local")
    cc_in = cc_in_t[:]
    cc_out = cc_out_t[:]
    nc.gpsimd.dma_start(cc_in, knTl[:])
    if NO_CC:
        for c in range(NCORES):
            nc.sync.dma_start(cc_out[c * SDK:(c + 1) * SDK, :], cc_in)
    else:
        nc.gpsimd.collective_compute(
            "AllGather", ALU.bypass,
            replica_groups=[list(range(NCORES))],
            ins=[cc_in.opt()], outs=[cc_out.opt()],
        )
    nc.scalar.dma_start(
        knT[:].rearrange("p (c n) -> p c n", c=NCORES),
        cc_out.rearrange("(c p) n -> p c n", p=SDK))

    emit_late_input_dmas()

    # ---------- queries [b, sdk] + normalize ----------
    for c in range(4):
        nc.tensor.matmul(q_ps,
                         zt_sb[:, c * BL:(c + 1) * BL],
                         wq_sb[:, c * SDK:(c + 1) * SDK],
                         start=(c == 0), stop=(c == 3))
    qsq = sb.tile([BL, S], F32, tag="qsq")
    qsc = ksc  # shared scratch, phases are sequential
    for s in range(S):
        nc.scalar.activation(qsc[:, s * DK:(s + 1) * DK],
                             q_ps[:, s * DK:(s + 1) * DK],
                             AF.Square,
                             accum_out=qsq[:, s:s + 1])
    qnorm = sb.tile([BL, S], F32, tag="qnorm")
    nc.scalar.activation(qnorm[:], qsq[:], AF.Sqrt)
    qrec = sb.tile([BL, S], F32, tag="qrec")
    nc.vector.tensor_scalar_add(qnorm[:], qnorm[:], 1e-8)
    nc.vector.reciprocal(qrec[:], qnorm[:])
    # fold softmax(aspect_logits) weights into q_n
    for s, w_s in ((0, w0_f), (1, w1_f)):
        nc.vector.tensor_scalar(
            q_n[:, s * DK:(s + 1) * DK],
            q_ps[:, s * DK:(s + 1) * DK],
            qrec[:, s:s + 1], float(w_s),
            op0=ALU.mult, op1=ALU.mult)
    nc.tensor.transpose(q_tp, q_n[:], identb[:])
    nc.scalar.activation(qnT[:], q_tp, AF.Copy)

    # ---------- h2 = z @ W_base^T ; hz = zb + gamma*h2 ----------
    for c in range(4):
        nc.tensor.matmul(h2_ps[:],
                         zt_sb[:, c * BL:(c + 1) * BL],
                         wbt_sb[:, c * D_MODEL:(c + 1) * D_MODEL],
                         start=(c == 0), stop=(c == 3))
    nc.vector.scalar_tensor_tensor(
        out=hz_sb[:], in0=h2_ps[:], scalar=float(gamma_f), in1=zb_sb[:],
        op0=ALU.mult, op1=ALU.add)

    # ---------- tT (pre-collective, under AG shadow) + scores ----------
    ex_sb = sb.tile([BL, N], BF16, tag="ex")
    sig_sb = sb.tile([BL, N], BF16, tag="sig")

    def emit_scores():
        for h in range(2):
            nc.tensor.matmul(scores_big[:, h * 512:(h + 1) * 512],
                             qnT[:], knT[:, h * 512:(h + 1) * 512],
                             start=True, stop=True)

    for r in range(R):
        for q4 in range(2):
            t_ps = ps.tile([BL, 512], F32, tag="t", bufs=4)
            for jj in range(4):
                j = q4 * 4 + jj
                for a in range(4):
                    nc.tensor.matmul(
                        t_ps[:, jj * 128:(jj + 1) * 128],
                        vt_sb[r][:, a * N + j * 128:a * N + (j + 1) * 128],
                        zt_sb[:, a * BL:(a + 1) * BL],
                        start=(a == 0), stop=(a == 3))
            g0 = (r * 8 + q4 * 4) * 128
            nc.scalar.activation(tT_sb[:, g0:g0 + 512], t_ps[:], AF.Copy)
    emit_scores()

    # ---------- threshold (top-16) + alpha ----------
    # DVE threshold chain emitted BEFORE the ACT sigmoid/exp: the tile
    # scheduler compresses cross-engine deps through the most recent
    # instructions, and the other order made MAX8 wait for EXP.
    m8a = sb.tile([BL, 8], F32, tag="m8a")
    nc.vector.max(out=m8a[:], in_=scores_big[:])
    s_mr = sb.tile([BL, N], F32, tag="s_mr")
    nc.vector.match_replace(out=s_mr[:], in_to_replace=m8a[:],
                            in_values=scores_big[:], imm_value=-1e30)
    m8b = sb.tile([BL, 8], F32, tag="m8b")
    nc.vector.max(out=m8b[:], in_=s_mr[:])
    nc.scalar.activation(sig_sb[:], scores_big[:], AF.Sigmoid,
                         scale=float(LAMBDA_SHARP),
                         bias=float(-LAMBDA_SHARP * tau_f))
    nc.scalar.activation(ex_sb[:], scores_big[:], AF.Exp)
    # pin a sqrt-table reload behind exp so the LN sqrt needs no load
    nc.scalar.activation(dum_out[:, 2:3], ex_sb[0:1, 0:1], AF.Sqrt)
    msig = sb.tile([BL, N], BF16, tag="msig")
    nc.vector.scalar_tensor_tensor(
        out=msig[:], in0=scores_big[:], scalar=m8b[:, 7:8], in1=sig_sb[:],
        op0=ALU.is_ge, op1=ALU.mult)
    alpha = sb.tile([BL, N], BF16, tag="alpha")
    den = sb.tile([BL, 1], F32, tag="den")
    nc.vector.scalar_tensor_tensor(
        out=alpha[:], in0=msig[:], scalar=1.0, in1=ex_sb[:],
        op0=ALU.mult, op1=ALU.mult, accum_out=den[:])
    # rdg = gamma / (SC_H * (den + 1e-8))
    den2 = sb.tile([BL, 1], F32, tag="den2")
    nc.vector.tensor_scalar(den2[:], den[:], float(SC_H / gamma_f),
                            float(SC_H * 1e-8 / gamma_f),
                            op0=ALU.mult, op1=ALU.add)
    rdg = sb.tile([BL, 1], F32, tag="rdg")
    nc.vector.reciprocal(rdg[:], den2[:])

    # ---------- alphaT (8 PE transposes -> fp8) ----------
    for j in range(8):
        nc.tensor.transpose(tp_ps[:, j * 128:(j + 1) * 128],
                            alpha[:, j * 128:(j + 1) * 128], identb[:])
    for j in range(8):
        nc.scalar.activation(alphaT[:, j * BL:(j + 1) * BL],
                             tp_ps[:, j * 128:(j + 1) * 128], AF.Copy)

    # ---------- sT = tT * alphaT (DVE, fp8 out); h1 DoubleRow ----------
    if USE_DR:
        # bias pairs (only need alphaT), then U pairs chase DVE
        for m in range(4):
            nc.tensor.matmul(
                h1_ps[:],
                alphaT[:, 2 * m * BL:(2 * m + 2) * BL]
                    .rearrange("p (two f) -> p two f", two=2),
                up_ap(64 + 2 * m, 2)
                    .rearrange("p (two f) -> p two f", two=2),
                start=(m == 0), stop=False, perf_mode=DR)
    else:
        for m in range(8):
            nc.tensor.matmul(h1_ps[:], alphaT[:, m * BL:(m + 1) * BL],
                             up_ap(64 + m, 1),
                             start=(m == 0), stop=False)
    for r in range(R):
        nc.vector.tensor_tensor(
            out=sT_sb[:, r * 1024:(r + 1) * 1024],
            in0=tT_sb[:, r * 1024:(r + 1) * 1024],
            in1=alphaT[:], op=ALU.mult)
        if USE_DR:
            for k in range(4):
                g = r * 8 + k * 2
                nc.tensor.matmul(
                    h1_ps[:],
                    sT_sb[:, g * 128:(g + 2) * 128]
                        .rearrange("p (two f) -> p two f", two=2),
                    up_ap(g, 2).rearrange("p (two f) -> p two f", two=2),
                    start=False, stop=(g == 62), perf_mode=DR)
        else:
            for k in range(8):
                g = r * 8 + k
                nc.tensor.matmul(
                    h1_ps[:], sT_sb[:, g * 128:(g + 1) * 128],
                    up_ap(g, 1), start=False, stop=(g == 63))

    # ---------- combine + layernorm (fused) ----------
    # x = h1*rdg + hz ; row-sum -> mean
    x_sb = sb.tile([BL, D_MODEL], F32, tag="x")
    xsum = sb.tile([BL, 1], F32, tag="xsum")
    nc.vector.scalar_tensor_tensor(
        out=x_sb[:], in0=h1_ps[:], scalar=rdg[:], in1=hz_sb[:],
        op0=ALU.mult, op1=ALU.add, accum_out=xsum[:])
    nmean = sb.tile([BL, 1], F32, tag="nmean")
    nc.vector.tensor_scalar_mul(nmean[:], xsum[:], -1.0 / D_MODEL)
    sq_scr = sb.tile([BL, D_MODEL], BF16, tag="sqscr2")
    ssq = sb.tile([BL, 1], F32, tag="ssq")
    nc.scalar.activation(sq_scr[:], x_sb[:], AF.Square, bias=nmean[:],
                         accum_out=ssq[:])
    vare = sb.tile([BL, 1], F32, tag="vare")
    nc.vector.tensor_scalar(vare[:], ssq[:], 1.0 / D_MODEL, LN_EPS,
                            op0=ALU.mult, op1=ALU.add)
    sd = sb.tile([BL, 1], F32, tag="sd")
    nc.scalar.activation(sd[:], vare[:], AF.Sqrt)
    rstd = sb.tile([BL, 1], F32, tag="rstd")
    nc.vector.reciprocal(rstd[:], sd[:])
    mrs = sb.tile([BL, 1], F32, tag="mrs")
    nc.vector.tensor_tensor(out=mrs[:], in0=nmean[:], in1=rstd[:],
                            op=ALU.mult)
    xn = sb.tile([BL, D_MODEL], F32, tag="xn")
    nc.vector.tensor_scalar(xn[:], x_sb[:], rstd[:], mrs[:],
                            op0=ALU.mult, op1=ALU.add)
    out_sb = x_sb  # dead
    nc.vector.tensor_tensor(out=out_sb[:], in0=xn[:], in1=ls_sb[:],
                            op=ALU.mult)
    nc.vector.tensor_tensor(out=out_sb[:], in0=out_sb[:], in1=lb_sb[:],
                            op=ALU.add)
    nc.scalar.dma_start(out_d[:], out_sb[:])


def _pack(x, p=128):
    """[K*p, F] row-chunked -> [p, K*F] (chunk k at cols k*F:(k+1)*F)."""
    k = x.shape[0] // p
    return np.ascontiguousarray(
        x.reshape(k, p, -1).transpose(1, 0, 2).reshape(p, -1))


def kernel(z, pool_vectors, W_Q, W_K, aspect_logits, tau,
           W_base, b_base, gamma, ln_scale, ln_bias):
    global LAST_EXEC_NS
    z = np.asarray(z, np.float32)
    pool = np.asarray(pool_vectors, np.float32)
    W_Q = np.asarray(W_Q, np.float32)
    W_K = np.asarray(W_K, np.float32)
    aspect_logits = np.asarray(aspect_logits, np.float32)
    tau_f = float(np.asarray(tau))
    W_base = np.asarray(W_base, np.float32)
    b_base = np.asarray(b_base, np.float32)
    gamma_f = float(np.asarray(gamma))
    ln_scale = np.asarray(ln_scale, np.float32)
    ln_bias = np.asarray(ln_bias, np.float32)

    e = np.exp(aspect_logits - aspect_logits.max())
    w = e / e.sum()
    w0_f, w1_f = float(w[0]), float(w[1])

    nc = _build(tau_f, w0_f, w1_f, gamma_f)

    fp8 = ml_dtypes.float8_e4m3
    bf16 = ml_dtypes.bfloat16

    # ---- shared host-side layout prep ----
    KCW = (128 // NKCH) * NL
    wk_cat = np.concatenate([W_K[0], W_K[1]], axis=1)          # [D, 128]
    wk = _pack((wk_cat * 64.0).astype(fp8))                    # [128, 128*128]
    wks = [np.ascontiguousarray(wk[:, c * KCW:(c + 1) * KCW])
           for c in range(NKCH)]
    wq = _pack(np.concatenate([W_Q[0], W_Q[1]], axis=1).astype(bf16))
    # V'^T in (r, n) order: vt_r[e, n] = SC_V * V[n, r, e]
    Vp = pool[:, U_END:V_END].reshape(N, R, D_MODEL)
    vts = []
    for r in range(R):
        vts.append(_pack(np.ascontiguousarray(
            Vp[:, r, :].T * SC_V).astype(fp8)))                # [128, 4*N]
    # up rows in (r, n) order: row r*N+n = SC_U * U[n, :, r];
    # bias rows (n order) scaled by SC_B
    Up = pool[:, :U_END].reshape(N, D_MODEL, R)
    up_rows = np.concatenate([
        np.ascontiguousarray(Up.transpose(2, 0, 1)).reshape(NR, D_MODEL)
        * SC_U,
        pool[:, V_END:B_END] * SC_B,
    ], axis=0)
    up = _pack(up_rows.astype(fp8))                            # [128, 72*512]
    ups = [np.ascontiguousarray(up[:, q * 18 * D_MODEL:(q + 1) * 18 * D_MODEL])
           for q in range(4)]
    wbt = _pack(np.ascontiguousarray(W_base.T).astype(bf16))   # [128, 4*512]
    ls = np.broadcast_to(ln_scale, (BL, D_MODEL)).astype(np.float32).copy()
    lb = np.broadcast_to(ln_bias, (BL, D_MODEL)).astype(np.float32).copy()
    gb = (gamma_f * b_base).astype(np.float32)

    in_maps = []
    for c in range(NCORES):
        z_loc = np.ascontiguousarray(z[c * BL:(c + 1) * BL])
        zt_loc = _pack(np.ascontiguousarray(z_loc.T).astype(fp8))
        pk_loc = _pack((np.ascontiguousarray(
            pool[c * NL:(c + 1) * NL, :].T) * 16.0).astype(fp8))
        m = {
            "wq": wq, "zt": zt_loc,
            "zb": z_loc + gb, "ls": ls, "lb": lb, "wbt": wbt,
        }
        for ch in range(NKCH):
            m[f"pk{ch}"] = np.ascontiguousarray(
                pk_loc[:, ch * KCW:(ch + 1) * KCW])
            m[f"wk{ch}"] = wks[ch]
        for r in range(R):
            m[f"vt{r}"] = vts[r]
        for q in range(4):
            m[f"up{q}"] = ups[q]
        in_maps.append(m)

    res = run_bass_kernel_spmd(nc, in_maps, core_ids=list(range(NCORES)),
                               trace=TRACE, trace_cores=TRACE_CORES,
                               tmpdir=TMPDIR)
    LAST_EXEC_NS = res.exec_time_ns
    out = np.concatenate([res.results[c]["out"] for c in range(NCORES)],
                         axis=0)
    return out.astype(np.float32)
